# revision 53
# baseline (speedup 1.0000x reference)
"""DeltaNet-style gated linear attention forward on 8 Trainium2 NeuronCores.

Sharding: core c = (batch b = c//4, sequence quarter r = c%4). Each core
projects q/k/v/gate for its 512 rows (all 16 heads), runs chunked linear
attention in quarter-local unscaled coordinates, exchanges per-quarter state
summaries via a small AllGather, then applies the inter-quarter state, output
projection, residual and LayerNorm for its own rows.

Math (per batch, head), matching the reference scan: with
b_i = prod_{j<=i} f_j (cumprod from quarter start), k~_j = phi_j / b_j,
the output row i inside a quarter is
  out_i = phi_i (S_start + U_i) / max(phi_i . (m_start + mU_i), eps)
(the q-side decay b_i cancels between numerator and denominator), where
U_i = sum_{j<=i, same quarter} k~_j v_j^T accumulates unscaled and
(S_start, m_start) is the true state entering the quarter, combined from the
peer quarters' summaries (A_q, A_q * U_q) after an AllGather.

Layouts: heads are processed in pairs p = (2p, 2p+1); D-major tensors put
head 2p on partitions 0:64 and head 2p+1 on partitions 64:128 so matmuls on
64-deep contractions / 64-wide outputs pack into PE array quadrants
(tile_position auto-derived from base partitions).

Schedule: pass A issues the k/v projections first on the PE queue and runs
the batched gate chain on DVE/scalar underneath them, so the per-chunk state
deltas U finish early and the AllGather fires early; pass B (q projection,
rope/phi, 2-head transposes, intra-chunk attention) overlaps the collective;
phase 2 (inter-chunk state application, output projection, residual,
LayerNorm) runs after. Output bias bo is folded into the residual rows
host-side.
"""

import numpy as np
import ml_dtypes

import bass_rust
import concourse.bass as bass
import concourse.mybir as mybir
import concourse.tile as tile
from concourse.bass_utils import run_bass_kernel_spmd

dt = mybir.dt
AF = mybir.ActivationFunctionType
ALU = mybir.AluOpType

B, T, C, H, D = 2, 2048, 1024, 16, 64
NCORE = 8
QT = T // 4          # rows per core
L = 128              # chunk length
NCH = QT // L        # chunks per core
KT = C // 128        # contraction tiles
NP = H // 2          # head pairs
SE = D + 1           # state row width (S | m)
ROPE_BASE = 10000.0
EPS = 1e-6
LN_EPS = 1e-5
G_CLAMP = -30.0      # per-chunk cumsum floor (defensive; inert for real data)
SPLIT_WAITS = True   # walrus here takes <=1 sem wait per instruction


# ---------------------------------------------------------------- walrus shim
def _split_multi_waits(nc):
    ctr = 0
    for fn in nc.m.functions:
        for bb in fn.blocks:
            out = []
            for ins in bb.instructions:
                si = ins.sync_info
                if si is not None and si.on_wait and len(si.on_wait) > 1:
                    waits = list(si.on_wait)
                    for w in waits[:-1]:
                        ctr += 1
                        nop = mybir.InstNoOp(name=f"WS-{ctr}", ins=[], outs=[])
                        nop.engine = ins.engine
                        nop.sync_info = bass_rust.SyncInfo(on_wait=[w], on_update=[])
                        nop.debug = ins.debug
                        out.append(nop)
                    si.on_wait = [waits[-1]]
                out.append(ins)
            bb.instructions[:] = out
    return ctr


def _register_const(nc, value, dtype=dt.float32):
    t = nc.alloc_sbuf_tensor(f"uconst-{dtype.name}-{value}", [128, 1], dtype)
    nc.gpsimd.memset(t.ap(), value)
    nc.const_aps.aps[(dtype, value)] = t.ap()


# ------------------------------------------------------------------- builder
def _enable_ldw_opt():
    try:
        from concourse.compiler_utils import get_compiler_flags, set_compiler_flags
        flags = get_compiler_flags()
        new = [f.replace("--enable-ldw-opt=false", "--enable-ldw-opt=true")
               for f in flags]
        if new != flags:
            set_compiler_flags(new)
    except Exception:
        pass


def build(has_mask=False, has_ln=False):
    _enable_ldw_opt()
    nc = bass.Bass(target_bir_lowering=False, debug=False)
    _register_const(nc, float(LN_EPS))
    nc.all_engine_barrier()

    f32 = dt.float32
    bf16 = dt.bfloat16
    P = {}

    def param(name, shape, dtype=f32, out=False):
        P[name] = nc.declare_dram_parameter(name, list(shape), dtype, isOutput=out)
        return P[name]

    param("xTb", (128, KT, QT), bf16)           # x rows^T (bf16)
    param("wg", (128, KT, H), bf16)             # Wg.T k-tiled
    param("wkv", (128, KT, 2 * C), bf16)        # [Wk.T|Wv.T] k-tiled
    param("wq", (128, KT, C), bf16)             # Wq.T k-tiled
    param("wo", (128, KT, C), bf16)             # Wo.T k-tiled
    param("xrows", (QT, C), bf16)              # residual rows (+bo folded)
    param("ropec", (128, NCH, 128))             # [cos|cos|-sin|+sin] per chunk
    param("triu", (128, 128))                   # j<=i ones (cumsum + causal)
    param("eye", (128, 128), bf16)              # PE transpose identity (bf16)
    param("eyef", (128, 128))                   # PE transpose identity (fp32)
    param("onesrow", (1, 128))
    param("negbg", (H, 1))                      # -bg column (gate exp bias)
    param("sel", (128, 4))                      # quarter-combine select (q < r)
    param("isel", (128, 4))                     # 1 - sel
    if has_mask:
        param("mkc", (128, NCH))
        param("mki", (128, NCH))
    if has_ln:
        param("lnw", (128, C))
        param("lnb", (128, C))
    param("y", (QT, C), bf16, out=True)

    one_ap = nc.const_aps.aps[(f32, 1.0)]
    CCW = NP * SE + NP   # collective cols: 8*65 S payload + 8 brun

    import contextlib
    with tile.TileContext(nc) as tc, contextlib.ExitStack() as outer:
        keep = outer.enter_context(tc.tile_pool(name="keep", bufs=1))
        wp = outer.enter_context(tc.tile_pool(name="wp", bufs=1))
        qtp = outer.enter_context(tc.tile_pool(name="qtp", bufs=NCH))
        nip = outer.enter_context(tc.tile_pool(name="nip", bufs=NCH))
        usp = outer.enter_context(tc.tile_pool(name="usp", bufs=NCH))
        ktp = outer.enter_context(tc.tile_pool(name="ktp", bufs=NCH))
        vxp = outer.enter_context(tc.tile_pool(name="vxp", bufs=NCH))
        xrp = outer.enter_context(tc.tile_pool(name="xrp", bufs=NCH))
        u16p = outer.enter_context(tc.tile_pool(name="u16p", bufs=NCH))
        dram = outer.enter_context(tc.tile_pool(name="dram", bufs=1, space="DRAM"))

        # bulk inputs in need-order: x + gate W first (the gate matmul is
        # the first PE work), then k/v W, gate consts, rope, q W, o W,
        # residuals
        triu_sb = keep.tile([128, 128], f32, name="triu_sb")
        eye_sb = keep.tile([128, 128], bf16, name="eye_sb")
        eyef_sb = keep.tile([128, 128], f32, name="eyef_sb")
        ones_row = keep.tile([1, 128], f32, name="ones_row")
        negbg_sb = keep.tile([H, 1], f32, name="negbg_sb")

        xtb_sb = wp.tile([128, KT, QT], bf16, name="xtb_sb")
        nc.sync.dma_start(xtb_sb[:], P["xTb"][:])
        wg_sb = wp.tile([128, KT, H], bf16, name="wg_sb")
        nc.sync.dma_start(wg_sb[:], P["wg"][:])
        nc.sync.dma_start(negbg_sb[:], P["negbg"][:])
        wkv_sb = wp.tile([128, KT, 2 * C], bf16, name="wkv_sb")
        for kt in range(KT):
            nc.sync.dma_start(wkv_sb[:, kt, :], P["wkv"][:, kt, :])
        for t_, p_ in ((triu_sb, "triu"), (eyef_sb, "eyef"),
                       (ones_row, "onesrow")):
            nc.sync.dma_start(t_[:], P[p_][:])
        if has_mask:
            mkc_sb = keep.tile([128, NCH], f32, name="mkc_sb")
            mki_sb = keep.tile([128, NCH], f32, name="mki_sb")
            nc.sync.dma_start(mkc_sb[:], P["mkc"][:])
            nc.sync.dma_start(mki_sb[:], P["mki"][:])
        rope_sb = keep.tile([128, NCH, 128], f32, name="rope_sb")
        nc.sync.dma_start(rope_sb[:], P["ropec"][:])
        nc.sync.dma_start(eye_sb[:], P["eye"][:])
        wq_sb = wp.tile([128, KT, C], bf16, name="wq_sb")
        for kt in range(KT):
            nc.sync.dma_start(wq_sb[:, kt, :], P["wq"][:, kt, :])
        sel_sb = keep.tile([128, 4], f32, name="sel_sb")
        isel_sb = keep.tile([128, 4], f32, name="isel_sb")
        nc.sync.dma_start(sel_sb[:], P["sel"][:])
        nc.sync.dma_start(isel_sb[:], P["isel"][:])
        wo_sb = wp.tile([128, KT, C], bf16, name="wo_sb")
        for kt in range(KT):
            nc.sync.dma_start(wo_sb[:, kt, :], P["wo"][:, kt, :])
        xr_tiles = []
        for ch in range(NCH):
            xr = xrp.tile([128, C], bf16, name=f"xr{ch}", tag="xr")
            nc.sync.dma_start(xr[:], P["xrows"][bass.ts(ch, 128), :])
            xr_tiles.append(xr)
        if has_ln:
            lnw_sb = wp.tile([128, C], f32, name="lnw_sb")
            lnb_sb = wp.tile([128, C], f32, name="lnb_sb")
            nc.sync.dma_start(lnw_sb[:], P["lnw"][:])
            nc.sync.dma_start(lnb_sb[:], P["lnb"][:])

        logfT = keep.tile([H, QT], f32, name="logfT")
        biall = keep.tile([128, NCH, H], bf16, name="biall")  # k decay scales

        ktm_tiles, vext_tiles, u_tiles, u16_tiles = [], [], [], []
        qt_tiles, ni_tiles = [], []

        # ------------------------------------------------- pass A
        with contextlib.ExitStack() as phA:
            pa = phA.enter_context(tc.tile_pool(name="pa", bufs=2, space="PSUM"))
            work = phA.enter_context(tc.tile_pool(name="work", bufs=2))
            small = phA.enter_context(tc.tile_pool(name="small", bufs=1))
            phA1 = phA.enter_context(contextlib.ExitStack())
            gp = phA1.enter_context(tc.tile_pool(name="gp", bufs=1, space="PSUM"))
            up = phA1.enter_context(tc.tile_pool(name="up", bufs=2, space="PSUM"))

            def kv_proj(ch, qmode=False, split=False):
                # k|v (or q) projections for one chunk; returns PSUM [128, C]
                # (k|v mode also computes v into a second PSUM tile; split
                # emits all-k then all-v so k's rope can start at half-block)
                lhss = xtb_sb[:, :, bass.ts(ch, L)]
                kps = pa.tile([128, C], f32, name="kps", tag="kv")
                vps = None if qmode else pa.tile([128, C], f32, name="vps",
                                                 tag="kv")
                wsrc = wq_sb if qmode else wkv_sb
                for kt in range(KT):
                    lhs = lhss[:, kt, :]
                    st, sp_ = (kt == 0), (kt == KT - 1)
                    nc.tensor.matmul(kps[:, 0:512], lhs,
                                     wsrc[:, kt, 0:512], start=st, stop=sp_)
                    nc.tensor.matmul(kps[:, 512:1024], lhs,
                                     wsrc[:, kt, 512:1024], start=st, stop=sp_)
                    if not qmode and not split:
                        nc.tensor.matmul(vps[:, 0:512], lhs,
                                         wkv_sb[:, kt, 1024:1536],
                                         start=st, stop=sp_)
                        nc.tensor.matmul(vps[:, 512:1024], lhs,
                                         wkv_sb[:, kt, 1536:2048],
                                         start=st, stop=sp_)
                if split:
                    for kt in range(KT):
                        lhs = lhss[:, kt, :]
                        st, sp_ = (kt == 0), (kt == KT - 1)
                        nc.tensor.matmul(vps[:, 0:512], lhs,
                                         wkv_sb[:, kt, 1024:1536],
                                         start=st, stop=sp_)
                        nc.tensor.matmul(vps[:, 512:1024], lhs,
                                         wkv_sb[:, kt, 1536:2048],
                                         start=st, stop=sp_)
                return kps, vps

            def rope_phi(ps, ch, tag):
                # copy PSUM to SBUF once (scalar) so the projection PSUM
                # frees fast, then rope + phi from SBUF on vector + gpsimd
                kcp = work.tile([128, H, D], bf16, name=f"kcp{tag}", tag="kcp")
                nc.scalar.copy(kcp[:], ps.rearrange("p (h d) -> p h d", h=H))
                view = kcp[:]
                rr = work.tile([128, H, D], bf16, name=f"rr{tag}", tag="rrk")
                tmp = work.tile([128, H, D], bf16, name=f"tmp{tag}", tag="tmpk")
                cs = rope_sb[:, ch, 0:64]
                sna = rope_sb[:, ch, 64:96]
                snb = rope_sb[:, ch, 96:128]
                nc.vector.tensor_mul(
                    rr[:], view, cs[:, None, :].to_broadcast([128, H, D]))
                nc.vector.tensor_mul(
                    tmp[:, :, 0:32], view[:, :, 32:64],
                    sna[:, None, :].to_broadcast([128, H, 32]))
                nc.vector.tensor_mul(
                    tmp[:, :, 32:64], view[:, :, 0:32],
                    snb[:, None, :].to_broadcast([128, H, 32]))
                rrf = rr.rearrange("p h d -> p (h d)")
                tmpf = tmp.rearrange("p h d -> p (h d)")
                nc.vector.tensor_add(rrf, rrf, tmpf)
                nc.vector.tensor_scalar_min(tmpf, rrf, 0.0)
                nc.scalar.activation(tmpf, tmpf, AF.Exp)
                nc.scalar.activation(rrf, rrf, AF.Relu)
                nc.vector.tensor_add(rrf, rrf, tmpf)
                return rr

            def kv_post(kps, vps, ch):
                # rope/phi/decay for k, ones-extended v, from the kv PSUM
                rr = rope_phi(kps, ch, "k")
                ktm_c = ktp.tile([128, H, D], bf16, name=f"ktm{ch}", tag="ktm")
                nc.vector.tensor_mul(
                    ktm_c[:], rr[:],
                    biall[:, ch, :, None].to_broadcast([128, H, D]))
                vext_c = vxp.tile([128, H, SE], bf16, name=f"vx{ch}", tag="vx")
                if has_mask:
                    nc.vector.tensor_mul(
                        vext_c[:, :, 0:D], vps.rearrange("p (h d) -> p h d", h=H),
                        mkc_sb[:, ch:ch + 1, None].to_broadcast([128, H, D]))
                else:
                    nc.scalar.copy(vext_c[:, :, 0:D],
                                   vps.rearrange("p (h d) -> p h d", h=H))
                nc.vector.tensor_copy(vext_c[:, :, D],
                                      nc.const_aps.aps[(bf16, 1.0)]
                                      .to_broadcast([128, H]))
                ktm_tiles.append(ktm_c)
                vext_tiles.append(vext_c)

            def u_delta(ch):
                # dU and cumulative U snapshot; the bf16 snapshot is
                # duplicated onto partitions 64:128 so phase-2 row-packed
                # matmuls can slice either half
                u_c = usp.tile([64, H, SE], f32, name=f"u{ch}", tag="u")
                for g in range(4):
                    ups = up.tile([64, 4, SE], f32, name="ups", tag="ups")
                    for j in range(4):
                        h = g * 4 + j
                        nc.tensor.matmul(ups[:, j, :], ktm_tiles[ch][:, h, :],
                                         vext_tiles[ch][:, h, :],
                                         start=(j == 0), stop=(j == 3))
                    dst = u_c[:, g * 4:(g + 1) * 4, :]
                    if ch == 0:
                        nc.vector.tensor_copy(dst, ups[:])
                    else:
                        nc.vector.tensor_add(
                            dst, ups[:], u_tiles[ch - 1][:, g * 4:(g + 1) * 4, :])
                u16_c = u16p.tile([64, H, SE], bf16, name=f"u16{ch}",
                                  tag="u16")
                nc.vector.tensor_copy(u16_c[:], u_c[:])
                u_tiles.append(u_c)
                u16_tiles.append(u16_c)

            # gate logits, transposed: gT[h, t] for the whole quarter
            gT = gp.tile([H, QT], f32, name="gT", tag="gbig")
            for kt in range(KT):
                nc.tensor.matmul(gT[:], wg_sb[:, kt, :], xtb_sb[:, kt, :],
                                 start=(kt == 0), stop=(kt == KT - 1))
            # f = clip(sigmoid(z+bg)) via exp (stays on the exp/ln act table)
            ef = small.tile([H, QT], f32, name="ef", tag="ef")
            nc.scalar.activation(ef[:], gT[:], AF.Exp,
                                 bias=negbg_sb[:], scale=-1.0)
            nc.vector.tensor_scalar_add(ef[:], ef[:], 1.0)
            nc.vector.reciprocal(ef[:], ef[:])
            nc.vector.tensor_scalar(ef[:], ef[:], 0.999, 0.01, ALU.min, ALU.max)
            nc.scalar.activation(logfT[:], ef[:], AF.Ln)

            # k chunk 0 keeps the PE busy while the gate DVE chain runs
            kvp = [None] * NCH
            kps0 = pa.tile([128, C], f32, name="kps", tag="kv")
            for kt in range(KT):
                lhs = xtb_sb[:, kt, bass.ts(0, L)]
                st, sp_ = (kt == 0), (kt == KT - 1)
                nc.tensor.matmul(kps0[:, 0:512], lhs,
                                 wkv_sb[:, kt, 0:512], start=st, stop=sp_)
                nc.tensor.matmul(kps0[:, 512:1024], lhs,
                                 wkv_sb[:, kt, 512:1024], start=st, stop=sp_)

            # ---- batched gate chain: per-chunk cumsums/totals in 7 matmuls
            lgp = gp.tile([128, NCH, H], f32, name="lgp", tag="gsm")
            for ch in range(NCH):
                nc.tensor.matmul(lgp[:, ch, :], logfT[:, bass.ts(ch, L)],
                                 eyef_sb[0:H, 0:H], is_transpose=True,
                                 start=(ch == 0), stop=(ch == NCH - 1))
            logf_sb = small.tile([128, NCH, H], f32, name="logf_sb", tag="logf")
            if has_mask:
                nc.vector.tensor_mul(
                    logf_sb[:], lgp[:],
                    mkc_sb[:, :, None].to_broadcast([128, NCH, H]))
            else:
                nc.vector.tensor_copy(logf_sb[:], lgp[:])
            logf_fl = logf_sb.rearrange("p c h -> p (c h)")
            gps = gp.tile([128, NCH * H], f32, name="gps", tag="gsm")
            nc.tensor.matmul(gps[:], triu_sb[:], logf_fl,
                             start=True, stop=True)
            glp = gp.tile([1, NCH * H], f32, name="glp", tag="gsm")
            nc.tensor.matmul(glp[:], one_ap[:, 0:1], logf_fl,
                             start=True, stop=True)
            # v chunk 0 on the PE while the gate scalar/vector chain runs
            vps0 = pa.tile([128, C], f32, name="vps", tag="kv")
            for kt in range(KT):
                lhs = xtb_sb[:, kt, bass.ts(0, L)]
                st, sp_ = (kt == 0), (kt == KT - 1)
                nc.tensor.matmul(vps0[:, 0:512], lhs,
                                 wkv_sb[:, kt, 1024:1536], start=st, stop=sp_)
                nc.tensor.matmul(vps0[:, 512:1024], lhs,
                                 wkv_sb[:, kt, 1536:2048], start=st, stop=sp_)
            a_c = small.tile([128, NCH, H], f32, name="a_c", tag="a_c")
            nc.vector.tensor_scalar_max(
                a_c.rearrange("p c h -> p (c h)"), gps[:], G_CLAMP)
            ainv = small.tile([128, NCH, H], f32, name="ainv", tag="ainv")
            nc.scalar.activation(ainv[:], a_c[:], AF.Exp, scale=-1.0)
            gl_sb = small.tile([1, 2, NCH * H], f32, name="gl_sb", tag="gl")
            nc.vector.tensor_scalar_max(gl_sb[:, 0, :], glp[:], G_CLAMP)
            egl = small.tile([1, 2, NCH * H], f32, name="egl", tag="egl")
            nc.scalar.activation(egl[:, 0, :], gl_sb[:, 0, :], AF.Exp)
            nc.scalar.activation(egl[:, 1, :], gl_sb[:, 0, :], AF.Exp,
                                 scale=-1.0)
            bca = gp.tile([128, 2 * NCH * H], f32, name="bca", tag="gsm")
            nc.tensor.matmul(bca[:], ones_row[:],
                             egl.rearrange("o s x -> o (s x)"),
                             start=True, stop=True)
            bca_sb = small.tile([128, 2 * NCH * H], f32, name="bca_sb",
                                tag="bca_sb")
            nc.vector.tensor_copy(bca_sb[:], bca[:])
            bfw = bca_sb.rearrange("p (s c h) -> p s c h", s=2, c=NCH)
            # running cross-chunk inverse products (clamped), k scales
            nc.scalar.copy(biall[:, 0, :], ainv[:, 0, :])
            nc.vector.tensor_mul(biall[:, 1, :], ainv[:, 1, :], bfw[:, 1, 0, :])
            bic = small.tile([128, 2, H], f32, name="bic", tag="bic")
            nc.vector.tensor_mul(bic[:, 0, :], bfw[:, 1, 0, :], bfw[:, 1, 1, :])
            nc.vector.tensor_scalar_min(bic[:, 0, :], bic[:, 0, :], 1e26)
            nc.vector.tensor_mul(biall[:, 2, :], ainv[:, 2, :], bic[:, 0, :])
            nc.vector.tensor_mul(bic[:, 1, :], bic[:, 0, :], bfw[:, 1, 2, :])
            nc.vector.tensor_scalar_min(bic[:, 1, :], bic[:, 1, :], 1e26)
            nc.vector.tensor_mul(biall[:, 3, :], ainv[:, 3, :], bic[:, 1, :])
            if has_mask:
                for ch in range(NCH):
                    nc.vector.tensor_mul(
                        biall[:, ch, :], biall[:, ch, :],
                        mkc_sb[:, ch:ch + 1].to_broadcast([128, H]))
            # full-quarter forward product brun; its bf16 copy goes into
            # the collective payload tile early, off the critical path
            br2 = small.tile([128, 2, H], f32, name="br2", tag="br2")
            nc.vector.tensor_mul(br2[:, 0, :], bfw[:, 0, 0, :], bfw[:, 0, 1, :])
            nc.vector.tensor_mul(br2[:, 1, :], bfw[:, 0, 2, :], bfw[:, 0, 3, :])
            brun = keep.tile([128, H], f32, name="brun")
            nc.vector.tensor_mul(brun[:], br2[:, 0, :], br2[:, 1, :])
            seffx = keep.tile([64, H * SE + H], bf16, name="seffx")
            nc.scalar.copy(seffx[:, H * SE:], brun[0:64, :])

            # ---- remaining chunks: kv proj / rope / state deltas interleaved
            kv_post(kps0, vps0, 0)
            kvp[1] = kv_proj(1)
            u_delta(0)
            kv_post(*kvp[1], 1)
            kvp[2] = kv_proj(2)
            u_delta(1)
            kv_post(*kvp[2], 2)
            kvp[3] = kv_proj(3, split=True)
            u_delta(2)
            kv_post(*kvp[3], 3)
            u_delta(3)

            # ------------------------------------- summary + AllGather (bf16)
            nc.vector.tensor_mul(
                seffx[:, 0:H * SE].rearrange("p (h e) -> p h e", h=H),
                u_tiles[-1][:],
                brun[0:64, :, None].to_broadcast([64, H, SE]))

            cc_in = dram.tile([64, H * SE + H], bf16, name="cc_in")
            cc_out = dram.tile([256, H * SE + H], bf16, name="cc_out")
            nc.sync.dma_start(cc_in[:], seffx[:])
            nc.gpsimd.collective_compute(
                "AllGather", ALU.bypass,
                replica_groups=[[0, 1, 2, 3], [4, 5, 6, 7]],
                ins=[cc_in.opt()], outs=[cc_out.opt()])
            peers_all = keep.tile([64, 4, H * SE + H], bf16, name="peers")
            nc.sync.dma_start(
                peers_all[:], cc_out.rearrange("(q p) c -> p q c", q=4))
            peers = [peers_all[:, q, :] for q in range(4)]

            # ------------------------- pass B: q, transposes, intra-chunk attn
            phA1.close()
            tp = phA.enter_context(tc.tile_pool(name="tp", bufs=2, space="PSUM"))
            ap_ = phA.enter_context(tc.tile_pool(name="ap", bufs=1, space="PSUM"))
            for ch in range(NCH):
                qps, _ = kv_proj(ch, qmode=True)
                rr = rope_phi(qps, ch, "q")
                rrf = rr.rearrange("p h d -> p (h d)")
                if has_mask:
                    nc.vector.tensor_mul(
                        rrf, rrf,
                        mkc_sb[:, ch:ch + 1].to_broadcast([128, H * D]))

                # transpose q (rr) and k (ktm) to D-major [64, H, 128]
                qt_c = qtp.tile([64, H, L], bf16, name=f"qt{ch}", tag="qt")
                kt_c = work.tile([64, H, L], bf16, name="kt_c", tag="kt_c")
                ktmf = ktm_tiles[ch].rearrange("p h d -> p (h d)")
                for which, (srcf, dst) in enumerate(((rrf, qt_c), (ktmf, kt_c))):
                    for tg in range(4):
                        tps = tp.tile([64, 4, L], bf16, name="tps", tag="tps")
                        for j in range(4):
                            h = tg * 4 + j
                            nc.tensor.matmul(
                                tps[:, j, :], srcf[:, bass.ts(h, 64)],
                                eye_sb[:], is_transpose=True,
                                start=(j == 0), stop=(j == 3))
                        if tg % 2 == 0:
                            nc.vector.tensor_copy(dst[:, tg * 4:(tg + 1) * 4, :],
                                                  tps[:])
                        else:
                            nc.scalar.copy(dst[:, tg * 4:(tg + 1) * 4, :], tps[:])

                # causal intra-chunk attention numerators
                ni_c = nip.tile([128, H, SE], bf16, name=f"ni{ch}", tag="ni")
                for g in range(4):
                    atp = ap_.tile([128, 512], f32, name="atp", tag="atp")
                    for j in range(4):
                        h = 4 * g + j
                        nc.tensor.matmul(atp[:, bass.ts(j, 128)],
                                         kt_c[:, h, :], qt_c[:, h, :],
                                         start=(j == 0), stop=(j == 3))
                    atm = work.tile([128, 4, 128], bf16, name="atm", tag="atm")
                    nc.vector.tensor_mul(
                        atm[:], atp.rearrange("p (j t) -> p j t", j=4),
                        triu_sb[:, None, :].to_broadcast([128, 4, 128]))
                    nps = ap_.tile([128, 4, SE], f32, name="nps", tag="nps")
                    for j in range(4):
                        h = 4 * g + j
                        nc.tensor.matmul(nps[:, j, :], atm[:, j, :],
                                         vext_tiles[ch][:, h, :],
                                         start=(j == 0),
                                         stop=(j == 3 and ch == 0))
                    if ch > 0:
                        # inter-chunk partial q @ U_{ch-1}: collective-free,
                        # so it runs here under the AllGather instead of in
                        # phase 2
                        for j in range(4):
                            h = 4 * g + j
                            nc.tensor.matmul(nps[:, j, :], qt_c[:, h, :],
                                             u16_tiles[ch - 1][:, h, :],
                                             start=False, stop=(j == 3))
                    if g % 2 == 0:
                        nc.scalar.copy(ni_c[:, 4 * g:4 * g + 4, :], nps[:])
                    else:
                        nc.vector.tensor_copy(ni_c[:, 4 * g:4 * g + 4, :],
                                              nps[:])
                qt_tiles.append(qt_c)
                ni_tiles.append(ni_c)

            # combine quarter states: sstart = sum_q S_q * W_q with
            # W_q = sel_q * prod_{q'>q} (A_q' sel_q' + 1-sel_q'); the bf16
            # result is duplicated onto partitions 64:128 for row packing
            sstart = keep.tile([64, H, SE], f32, name="sstart")
            sst16 = keep.tile([64, H, SE], bf16, name="sst16")
            prodP = small.tile([64, H], f32, name="prodP", tag="prodP")
            nc.vector.tensor_copy(prodP[:], one_ap[0:64].to_broadcast([64, H]))
            pv = {}
            for q in (3, 2, 1, 0):
                wq_ = small.tile([64, H], f32, name=f"wq{q}", tag=f"wq{q}")
                nc.vector.tensor_mul(
                    wq_[:], prodP[:],
                    sel_sb[0:64, q:q + 1].to_broadcast([64, H]))
                pv[q] = wq_
                if q:
                    mq = small.tile([64, H], f32, name=f"mq{q}", tag="mq")
                    nc.vector.tensor_mul(
                        mq[:], peers[q][:, H * SE:],
                        sel_sb[0:64, q:q + 1].to_broadcast([64, H]))
                    nc.vector.tensor_add(
                        mq[:], mq[:],
                        isel_sb[0:64, q:q + 1].to_broadcast([64, H]))
                    nc.vector.tensor_mul(prodP[:], prodP[:], mq[:])
            tmon = work.tile([64, H, SE], f32, name="tmon", tag="kt_c")
            tmtw = work.tile([64, H, SE], f32, name="tmtw", tag="rrk")
            nc.vector.tensor_mul(
                tmon[:],
                peers[0][:, 0:H * SE].rearrange("p (h e) -> p h e", h=H),
                pv[0][:, :, None].to_broadcast([64, H, SE]))
            nc.vector.tensor_mul(
                tmtw[:],
                peers[1][:, 0:H * SE].rearrange("p (h e) -> p h e", h=H),
                pv[1][:, :, None].to_broadcast([64, H, SE]))
            nc.vector.tensor_add(sstart[:], tmon[:], tmtw[:])
            nc.vector.tensor_mul(
                tmon[:],
                peers[2][:, 0:H * SE].rearrange("p (h e) -> p h e", h=H),
                pv[2][:, :, None].to_broadcast([64, H, SE]))
            nc.vector.tensor_mul(
                tmtw[:],
                peers[3][:, 0:H * SE].rearrange("p (h e) -> p h e", h=H),
                pv[3][:, :, None].to_broadcast([64, H, SE]))
            nc.vector.tensor_add(tmon[:], tmon[:], tmtw[:])
            nc.vector.tensor_add(sstart[:], sstart[:], tmon[:])
            nc.scalar.copy(sst16[:], sstart[:])

        # ---------------------------------------------------------- phase 2
        with contextlib.ExitStack() as ph2:
            wk2 = ph2.enter_context(tc.tile_pool(name="wk2", bufs=2))
            att = ph2.enter_context(tc.tile_pool(name="att", bufs=NCH))
            ip = ph2.enter_context(tc.tile_pool(name="ip", bufs=3, space="PSUM"))
            tp2 = ph2.enter_context(tc.tile_pool(name="tp2", bufs=1, space="PSUM"))
            op = ph2.enter_context(tc.tile_pool(name="op", bufs=2, space="PSUM"))

            # inter-chunk numerators for all chunks first (keeps PE dense);
            # the intra-chunk term ni joins via a DVE add out of PSUM
            attns = []
            for ch in range(NCH):
                attn = att.tile([128, H, D], bf16, name=f"attn{ch}", tag="attn")
                nif = ni_tiles[ch].rearrange("p h e -> p (h e)")
                for g in range(4):
                    ips = ip.tile([128, 4, SE], f32, name="ips", tag="ips")
                    for j in range(4):
                        h = 4 * g + j
                        nc.tensor.matmul(ips[:, j, :], qt_tiles[ch][:, h, :],
                                         sst16[:, h, :],
                                         start=(j == 0), stop=False)
                    nc.tensor.matmul(ips.rearrange("p j e -> p (j e)"),
                                     eye_sb[:],
                                     nif[:, g * 4 * SE:(g + 1) * 4 * SE],
                                     start=False, stop=True)
                    sl4 = slice(4 * g, 4 * g + 4)
                    rden = wk2.tile([128, 4], f32, name="rden", tag="rden")
                    nc.vector.tensor_scalar_max(rden[:], ips[:, :, D], EPS)
                    nc.vector.reciprocal(rden[:], rden[:])
                    nc.vector.tensor_mul(
                        attn[:, sl4, :], ips[:, :, 0:D],
                        rden[:, :, None].to_broadcast([128, 4, D]))
                attns.append(attn)

            for ch in range(NCH):
                # transpose attn -> C-major, then o-proj + residual(+bias)
                attf = attns[ch].rearrange("p h d -> p (h d)")
                at_sb = wk2.tile([128, KT, 128], bf16, name="at_sb", tag="at_sb")
                for tg in range(2):
                    tps = tp2.tile([128, 512], bf16, name="tps2", tag="tps2")
                    for j in range(4):
                        nc.tensor.matmul(
                            tps[:, bass.ts(j, 128)],
                            attf[:, bass.ts(tg * 4 + j, 128)],
                            eye_sb[:], is_transpose=True,
                            start=(j == 0), stop=(j == 3))
                    if tg == 0:
                        nc.vector.tensor_copy(
                            at_sb.rearrange("p k t -> p (k t)")[:, 0:512], tps[:])
                    else:
                        nc.scalar.copy(
                            at_sb.rearrange("p k t -> p (k t)")[:, 512:1024],
                            tps[:])

                ops = op.tile([128, C], f32, name="ops", tag="ops")
                for nh in range(2):
                    nsl = bass.ts(nh, 512)
                    for kt in range(KT):
                        nc.tensor.matmul(ops[:, nsl], at_sb[:, kt, :],
                                         wo_sb[:, kt, nsl],
                                         start=(kt == 0), stop=(kt == KT - 1))

                # residual add off-PE, frees the o-proj PSUM with one reader
                osum = wk2.tile([128, C], f32, name="osum", tag="osum")
                nc.vector.tensor_add(osum[:], ops[:], xr_tiles[ch][:])

                # LayerNorm (Identity/Square/Sqrt live in one act table)
                mus = wk2.tile([128, 4], f32, name="mus", tag="mus")
                scr = wk2.tile([128, C], f32, name="scr", tag="scr")
                nc.scalar.activation(scr[:], osum[:], AF.Identity,
                                     accum_out=mus[:, 0:1])
                nc.vector.tensor_scalar_mul(mus[:, 1:2], mus[:, 0:1], -1.0 / C)
                nc.scalar.activation(scr[:], osum[:], AF.Square,
                                     bias=mus[:, 1:2], accum_out=mus[:, 2:3])
                nc.vector.tensor_scalar(mus[:, 2:3], mus[:, 2:3], 1.0 / C,
                                        LN_EPS, ALU.mult, ALU.add)
                nc.scalar.activation(mus[:, 2:3], mus[:, 2:3], AF.Sqrt)
                nc.vector.reciprocal(mus[:, 2:3], mus[:, 2:3])
                nc.vector.tensor_mul(mus[:, 3:4], mus[:, 1:2], mus[:, 2:3])
                yln = wk2.tile([128, C], bf16, name="yln", tag="scr")
                nc.scalar.activation(yln[:], osum[:], AF.Identity,
                                     scale=mus[:, 2:3], bias=mus[:, 3:4])
                if has_ln:
                    nc.vector.tensor_mul(yln[:], yln[:], lnw_sb[:])
                    nc.vector.tensor_add(yln[:], yln[:], lnb_sb[:])
                nc.sync.dma_start(P["y"][bass.ts(ch, 128), :], yln[:])

    if SPLIT_WAITS:
        _split_multi_waits(nc)
    return nc


# ---------------------------------------------------------------- host side
def _rope_tables():
    half = D // 2
    inv = 1.0 / (ROPE_BASE ** (np.arange(half, dtype=np.float64) / half))
    t = np.arange(T, dtype=np.float64)
    fr = t[:, None] * inv[None, :]
    cos, sin = np.cos(fr), np.sin(fr)
    out = np.zeros((T, 128), np.float32)
    out[:, 0:32] = cos
    out[:, 32:64] = cos
    out[:, 64:96] = -sin
    out[:, 96:128] = sin
    return out


def _ktile(w, dtype=np.float32):  # [C, N] -> [128, KT, N]
    return np.ascontiguousarray(
        w.reshape(KT, 128, w.shape[1]).transpose(1, 0, 2)).astype(dtype)


_cache = {}
RUN_KW = {}      # extra kwargs for run_bass_kernel_spmd (test harness profiling)
LAST = None      # last BassKernelResults (test harness reads exec_time_ns)


def kernel(x, mask, Wq, Wk, Wv, Wg, bg, Wo, bo, ln_w, ln_b):
    bfl = ml_dtypes.bfloat16
    x = np.asarray(x, np.float32)
    mask = np.asarray(mask)
    has_mask = not np.all(mask == 1)
    has_ln = not (np.all(np.asarray(ln_w) == 1) and np.all(np.asarray(ln_b) == 0))

    key = (has_mask, has_ln)
    if key not in _cache:
        _cache[key] = build(has_mask, has_ln)
    nc = _cache[key]

    wkv = _ktile(np.concatenate(
        [np.asarray(Wk).T, np.asarray(Wv).T], axis=1), bfl)
    wq_t = _ktile(np.ascontiguousarray(np.asarray(Wq).T), bfl)
    wg_t = _ktile(np.ascontiguousarray(np.asarray(Wg, np.float32).T), bfl)
    wo_t = _ktile(np.ascontiguousarray(np.asarray(Wo).T), bfl)
    ropec_full = _rope_tables()
    triu = np.triu(np.ones((128, 128), np.float32))
    eye = np.eye(128)
    onesrow = np.ones((1, 128), np.float32)
    negbg = -np.asarray(bg, np.float32)[:, None]
    bo_f = np.asarray(bo, np.float32)

    in_maps = []
    for c in range(NCORE):
        b, r = c // 4, c % 4
        rows = slice(r * QT, (r + 1) * QT)
        xq = np.ascontiguousarray(x[b, rows].T)   # [C, QT]
        m = {
            "xTb": _ktile(xq, bfl),
            "wg": wg_t,
            "wkv": wkv,
            "wq": wq_t,
            "wo": wo_t,
            "xrows": (np.ascontiguousarray(x[b, rows]) + bo_f[None, :]).astype(bfl),
            "ropec": np.ascontiguousarray(
                ropec_full[rows].reshape(NCH, 128, 128).transpose(1, 0, 2)),
            "triu": triu,
            "eye": eye.astype(bfl),
            "eyef": eye.astype(np.float32),
            "onesrow": onesrow,
            "negbg": negbg,
        }
        sel = np.zeros((128, 4), np.float32)
        sel[:, 0:r] = 1.0
        m["sel"] = sel
        m["isel"] = 1.0 - sel
        if has_mask:
            mk = np.asarray(mask[b, rows], np.float32)
            m["mkc"] = np.ascontiguousarray(mk.reshape(NCH, 128).T)
            m["mki"] = 1.0 - m["mkc"]
        if has_ln:
            m["lnw"] = np.tile(np.asarray(ln_w, np.float32), (128, 1))
            m["lnb"] = np.tile(np.asarray(ln_b, np.float32), (128, 1))
        in_maps.append(m)

    res = run_bass_kernel_spmd(nc, in_maps, list(range(NCORE)), **RUN_KW)
    globals()["LAST"] = res
    out = np.empty((B, T, C), np.float32)
    for c in range(NCORE):
        b, r = c // 4, c % 4
        out[b, r * QT:(r + 1) * QT, :] = res.results[c]["y"]
    return out


# revision 54
# speedup vs baseline: 1.0261x; 1.0261x over previous
"""DeltaNet-style gated linear attention forward on 8 Trainium2 NeuronCores.

Sharding: core c = (batch b = c//4, sequence quarter r = c%4). Each core
projects q/k/v/gate for its 512 rows (all 16 heads), runs chunked linear
attention in quarter-local unscaled coordinates, exchanges per-quarter state
summaries via a small AllGather, then applies the inter-quarter state, output
projection, residual and LayerNorm for its own rows.

Math (per batch, head), matching the reference scan: with
b_i = prod_{j<=i} f_j (cumprod from quarter start), k~_j = phi_j / b_j,
the output row i inside a quarter is
  out_i = phi_i (S_start + U_i) / max(phi_i . (m_start + mU_i), eps)
(the q-side decay b_i cancels between numerator and denominator), where
U_i = sum_{j<=i, same quarter} k~_j v_j^T accumulates unscaled and
(S_start, m_start) is the true state entering the quarter, combined from the
peer quarters' summaries (A_q, A_q * U_q) after an AllGather.

Layouts: heads are processed in pairs p = (2p, 2p+1); D-major tensors put
head 2p on partitions 0:64 and head 2p+1 on partitions 64:128 so matmuls on
64-deep contractions / 64-wide outputs pack into PE array quadrants
(tile_position auto-derived from base partitions).

Schedule: pass A issues the k/v projections first on the PE queue and runs
the batched gate chain on DVE/scalar underneath them, so the per-chunk state
deltas U finish early and the AllGather fires early; pass B (q projection,
rope/phi, 2-head transposes, intra-chunk attention) overlaps the collective;
phase 2 (inter-chunk state application, output projection, residual,
LayerNorm) runs after. Output bias bo is folded into the residual rows
host-side.
"""

import numpy as np
import ml_dtypes

import bass_rust
import concourse.bass as bass
import concourse.mybir as mybir
import concourse.tile as tile
from concourse.bass_utils import run_bass_kernel_spmd

dt = mybir.dt
AF = mybir.ActivationFunctionType
ALU = mybir.AluOpType

B, T, C, H, D = 2, 2048, 1024, 16, 64
NCORE = 8
QT = T // 4          # rows per core
L = 128              # chunk length
NCH = QT // L        # chunks per core
KT = C // 128        # contraction tiles
NP = H // 2          # head pairs
SE = D + 1           # state row width (S | m)
ROPE_BASE = 10000.0
EPS = 1e-6
LN_EPS = 1e-5
G_CLAMP = -30.0      # per-chunk cumsum floor (defensive; inert for real data)
SPLIT_WAITS = True   # walrus here takes <=1 sem wait per instruction


# ---------------------------------------------------------------- walrus shim
def _split_multi_waits(nc):
    ctr = 0
    for fn in nc.m.functions:
        for bb in fn.blocks:
            out = []
            for ins in bb.instructions:
                si = ins.sync_info
                if si is not None and si.on_wait and len(si.on_wait) > 1:
                    waits = list(si.on_wait)
                    for w in waits[:-1]:
                        ctr += 1
                        nop = mybir.InstNoOp(name=f"WS-{ctr}", ins=[], outs=[])
                        nop.engine = ins.engine
                        nop.sync_info = bass_rust.SyncInfo(on_wait=[w], on_update=[])
                        nop.debug = ins.debug
                        out.append(nop)
                    si.on_wait = [waits[-1]]
                out.append(ins)
            bb.instructions[:] = out
    return ctr


def _register_const(nc, value, dtype=dt.float32):
    t = nc.alloc_sbuf_tensor(f"uconst-{dtype.name}-{value}", [128, 1], dtype)
    nc.gpsimd.memset(t.ap(), value)
    nc.const_aps.aps[(dtype, value)] = t.ap()


# ------------------------------------------------------------------- builder
def _enable_ldw_opt():
    try:
        from concourse.compiler_utils import get_compiler_flags, set_compiler_flags
        flags = get_compiler_flags()
        new = [f.replace("--enable-ldw-opt=false", "--enable-ldw-opt=true")
               for f in flags]
        if new != flags:
            set_compiler_flags(new)
    except Exception:
        pass


def build(has_mask=False, has_ln=False):
    _enable_ldw_opt()
    nc = bass.Bass(target_bir_lowering=False, debug=False)
    _register_const(nc, float(LN_EPS))
    nc.all_engine_barrier()

    f32 = dt.float32
    bf16 = dt.bfloat16
    P = {}

    def param(name, shape, dtype=f32, out=False):
        P[name] = nc.declare_dram_parameter(name, list(shape), dtype, isOutput=out)
        return P[name]

    param("xTb", (128, KT, QT), bf16)           # x rows^T (bf16)
    param("wg", (128, KT, H), bf16)             # Wg.T k-tiled
    param("wkv", (128, KT, 2 * C), bf16)        # [Wk.T|Wv.T] k-tiled
    param("wq", (128, KT, C), bf16)             # Wq.T k-tiled
    param("wo", (128, KT, C), bf16)             # Wo.T k-tiled
    param("xrows", (QT, C), bf16)              # residual rows (+bo folded)
    param("ropec", (128, NCH, 128))             # [cos|cos|-sin|+sin] per chunk
    param("triu", (128, 128))                   # j<=i ones (cumsum + causal)
    param("eye", (128, 128), bf16)              # PE transpose identity (bf16)
    param("eyef", (128, 128))                   # PE transpose identity (fp32)
    param("onesrow", (1, 128))
    param("negbg", (H, 1))                      # -bg column (gate exp bias)
    param("sel", (128, 4))                      # quarter-combine select (q < r)
    param("isel", (128, 4))                     # 1 - sel
    if has_mask:
        param("mkc", (128, NCH))
        param("mki", (128, NCH))
    if has_ln:
        param("lnw", (128, C))
        param("lnb", (128, C))
    param("y", (QT, C), bf16, out=True)

    one_ap = nc.const_aps.aps[(f32, 1.0)]
    CCW = NP * SE + NP   # collective cols: 8*65 S payload + 8 brun

    import contextlib
    with tile.TileContext(nc) as tc, contextlib.ExitStack() as outer:
        keep = outer.enter_context(tc.tile_pool(name="keep", bufs=1))
        wp = outer.enter_context(tc.tile_pool(name="wp", bufs=1))
        qtp = outer.enter_context(tc.tile_pool(name="qtp", bufs=NCH))
        nip = outer.enter_context(tc.tile_pool(name="nip", bufs=NCH))
        usp = outer.enter_context(tc.tile_pool(name="usp", bufs=NCH))
        ktp = outer.enter_context(tc.tile_pool(name="ktp", bufs=NCH))
        vxp = outer.enter_context(tc.tile_pool(name="vxp", bufs=NCH))
        xrp = outer.enter_context(tc.tile_pool(name="xrp", bufs=NCH))
        u16p = outer.enter_context(tc.tile_pool(name="u16p", bufs=NCH))
        dram = outer.enter_context(tc.tile_pool(name="dram", bufs=1, space="DRAM"))

        # bulk inputs in need-order: x + gate W first (the gate matmul is
        # the first PE work), then k/v W, gate consts, rope, q W, o W,
        # residuals
        triu_sb = keep.tile([128, 128], f32, name="triu_sb")
        eye_sb = keep.tile([128, 128], bf16, name="eye_sb")
        eyef_sb = keep.tile([128, 128], f32, name="eyef_sb")
        ones_row = keep.tile([1, 128], f32, name="ones_row")
        negbg_sb = keep.tile([H, 1], f32, name="negbg_sb")

        xtb_sb = wp.tile([128, KT, QT], bf16, name="xtb_sb")
        nc.sync.dma_start(xtb_sb[:], P["xTb"][:])
        wg_sb = wp.tile([128, KT, H], bf16, name="wg_sb")
        nc.sync.dma_start(wg_sb[:], P["wg"][:])
        nc.sync.dma_start(negbg_sb[:], P["negbg"][:])
        wkv_sb = wp.tile([128, KT, 2 * C], bf16, name="wkv_sb")
        for kt in range(KT):
            nc.sync.dma_start(wkv_sb[:, kt, :], P["wkv"][:, kt, :])
        for t_, p_ in ((triu_sb, "triu"), (eyef_sb, "eyef"),
                       (ones_row, "onesrow")):
            nc.sync.dma_start(t_[:], P[p_][:])
        if has_mask:
            mkc_sb = keep.tile([128, NCH], f32, name="mkc_sb")
            mki_sb = keep.tile([128, NCH], f32, name="mki_sb")
            nc.sync.dma_start(mkc_sb[:], P["mkc"][:])
            nc.sync.dma_start(mki_sb[:], P["mki"][:])
        rope_sb = keep.tile([128, NCH, 128], f32, name="rope_sb")
        nc.sync.dma_start(rope_sb[:], P["ropec"][:])
        nc.sync.dma_start(eye_sb[:], P["eye"][:])
        wq_sb = wp.tile([128, KT, C], bf16, name="wq_sb")
        for kt in range(KT):
            nc.sync.dma_start(wq_sb[:, kt, :], P["wq"][:, kt, :])
        sel_sb = keep.tile([128, 4], f32, name="sel_sb")
        isel_sb = keep.tile([128, 4], f32, name="isel_sb")
        nc.sync.dma_start(sel_sb[:], P["sel"][:])
        nc.sync.dma_start(isel_sb[:], P["isel"][:])
        wo_sb = wp.tile([128, KT, C], bf16, name="wo_sb")
        for kt in range(KT):
            nc.sync.dma_start(wo_sb[:, kt, :], P["wo"][:, kt, :])
        xr_tiles = []
        for ch in range(NCH):
            xr = xrp.tile([128, C], bf16, name=f"xr{ch}", tag="xr")
            nc.sync.dma_start(xr[:], P["xrows"][bass.ts(ch, 128), :])
            xr_tiles.append(xr)
        if has_ln:
            lnw_sb = wp.tile([128, C], f32, name="lnw_sb")
            lnb_sb = wp.tile([128, C], f32, name="lnb_sb")
            nc.sync.dma_start(lnw_sb[:], P["lnw"][:])
            nc.sync.dma_start(lnb_sb[:], P["lnb"][:])

        logfT = keep.tile([H, QT], f32, name="logfT")
        biall = keep.tile([128, NCH, H], bf16, name="biall")  # k decay scales

        ktm_tiles, vext_tiles, u_tiles, u16_tiles = [], [], [], []
        qt_tiles, ni_tiles = [], []

        # ------------------------------------------------- pass A
        with contextlib.ExitStack() as phA:
            pa = phA.enter_context(tc.tile_pool(name="pa", bufs=2, space="PSUM"))
            work = phA.enter_context(tc.tile_pool(name="work", bufs=2))
            small = phA.enter_context(tc.tile_pool(name="small", bufs=1))
            phA1 = phA.enter_context(contextlib.ExitStack())
            gp = phA1.enter_context(tc.tile_pool(name="gp", bufs=1, space="PSUM"))
            up = phA1.enter_context(tc.tile_pool(name="up", bufs=2, space="PSUM"))

            def kv_proj(ch, qmode=False, split=False):
                # k|v (or q) projections for one chunk; returns PSUM [128, C]
                # (k|v mode also computes v into a second PSUM tile; split
                # emits all-k then all-v so k's rope can start at half-block)
                lhss = xtb_sb[:, :, bass.ts(ch, L)]
                kps = pa.tile([128, C], f32, name="kps", tag="kv")
                vps = None if qmode else pa.tile([128, C], f32, name="vps",
                                                 tag="kv")
                wsrc = wq_sb if qmode else wkv_sb
                for kt in range(KT):
                    lhs = lhss[:, kt, :]
                    st, sp_ = (kt == 0), (kt == KT - 1)
                    nc.tensor.matmul(kps[:, 0:512], lhs,
                                     wsrc[:, kt, 0:512], start=st, stop=sp_)
                    nc.tensor.matmul(kps[:, 512:1024], lhs,
                                     wsrc[:, kt, 512:1024], start=st, stop=sp_)
                    if not qmode and not split:
                        nc.tensor.matmul(vps[:, 0:512], lhs,
                                         wkv_sb[:, kt, 1024:1536],
                                         start=st, stop=sp_)
                        nc.tensor.matmul(vps[:, 512:1024], lhs,
                                         wkv_sb[:, kt, 1536:2048],
                                         start=st, stop=sp_)
                if split:
                    for kt in range(KT):
                        lhs = lhss[:, kt, :]
                        st, sp_ = (kt == 0), (kt == KT - 1)
                        nc.tensor.matmul(vps[:, 0:512], lhs,
                                         wkv_sb[:, kt, 1024:1536],
                                         start=st, stop=sp_)
                        nc.tensor.matmul(vps[:, 512:1024], lhs,
                                         wkv_sb[:, kt, 1536:2048],
                                         start=st, stop=sp_)
                return kps, vps

            def rope_phi(ps, ch, tag):
                # copy PSUM to SBUF once (scalar) so the projection PSUM
                # frees fast, then rope + phi from SBUF on vector + gpsimd
                kcp = work.tile([128, H, D], bf16, name=f"kcp{tag}", tag="kcp")
                nc.scalar.copy(kcp[:], ps.rearrange("p (h d) -> p h d", h=H))
                view = kcp[:]
                rr = work.tile([128, H, D], bf16, name=f"rr{tag}", tag="rrk")
                tmp = work.tile([128, H, D], bf16, name=f"tmp{tag}", tag="tmpk")
                cs = rope_sb[:, ch, 0:64]
                sna = rope_sb[:, ch, 64:96]
                snb = rope_sb[:, ch, 96:128]
                nc.vector.tensor_mul(
                    rr[:], view, cs[:, None, :].to_broadcast([128, H, D]))
                nc.vector.tensor_mul(
                    tmp[:, :, 0:32], view[:, :, 32:64],
                    sna[:, None, :].to_broadcast([128, H, 32]))
                nc.vector.tensor_mul(
                    tmp[:, :, 32:64], view[:, :, 0:32],
                    snb[:, None, :].to_broadcast([128, H, 32]))
                rrf = rr.rearrange("p h d -> p (h d)")
                tmpf = tmp.rearrange("p h d -> p (h d)")
                nc.vector.tensor_add(rrf, rrf, tmpf)
                nc.vector.tensor_scalar_min(tmpf, rrf, 0.0)
                nc.scalar.activation(tmpf, tmpf, AF.Exp)
                nc.scalar.activation(rrf, rrf, AF.Relu)
                nc.vector.tensor_add(rrf, rrf, tmpf)
                return rr

            def kv_post(kps, vps, ch):
                # rope/phi/decay for k, ones-extended v, from the kv PSUM
                rr = rope_phi(kps, ch, "k")
                ktm_c = ktp.tile([128, H, D], bf16, name=f"ktm{ch}", tag="ktm")
                nc.vector.tensor_mul(
                    ktm_c[:], rr[:],
                    biall[:, ch, :, None].to_broadcast([128, H, D]))
                vext_c = vxp.tile([128, H, SE], bf16, name=f"vx{ch}", tag="vx")
                if has_mask:
                    nc.vector.tensor_mul(
                        vext_c[:, :, 0:D], vps.rearrange("p (h d) -> p h d", h=H),
                        mkc_sb[:, ch:ch + 1, None].to_broadcast([128, H, D]))
                else:
                    nc.scalar.copy(vext_c[:, :, 0:D],
                                   vps.rearrange("p (h d) -> p h d", h=H))
                nc.vector.tensor_copy(vext_c[:, :, D],
                                      nc.const_aps.aps[(bf16, 1.0)]
                                      .to_broadcast([128, H]))
                ktm_tiles.append(ktm_c)
                vext_tiles.append(vext_c)

            def u_delta(ch):
                # dU and cumulative U snapshot; the bf16 snapshot is
                # duplicated onto partitions 64:128 so phase-2 row-packed
                # matmuls can slice either half
                u_c = usp.tile([64, H, SE], f32, name=f"u{ch}", tag="u")
                for g in range(4):
                    ups = up.tile([64, 4, SE], f32, name="ups", tag="ups")
                    for j in range(4):
                        h = g * 4 + j
                        nc.tensor.matmul(ups[:, j, :], ktm_tiles[ch][:, h, :],
                                         vext_tiles[ch][:, h, :],
                                         start=(j == 0), stop=(j == 3))
                    dst = u_c[:, g * 4:(g + 1) * 4, :]
                    if ch == 0:
                        nc.vector.tensor_copy(dst, ups[:])
                    else:
                        nc.vector.tensor_add(
                            dst, ups[:], u_tiles[ch - 1][:, g * 4:(g + 1) * 4, :])
                u16_c = u16p.tile([64, H, SE], bf16, name=f"u16{ch}",
                                  tag="u16")
                nc.vector.tensor_copy(u16_c[:], u_c[:])
                u_tiles.append(u_c)
                u16_tiles.append(u16_c)

            # gate logits, transposed: gT[h, t] for the whole quarter
            gT = gp.tile([H, QT], f32, name="gT", tag="gbig")
            for kt in range(KT):
                nc.tensor.matmul(gT[:], wg_sb[:, kt, :], xtb_sb[:, kt, :],
                                 start=(kt == 0), stop=(kt == KT - 1))
            # f = clip(sigmoid(z+bg)) via exp (stays on the exp/ln act table)
            ef = small.tile([H, QT], f32, name="ef", tag="ef")
            nc.scalar.activation(ef[:], gT[:], AF.Exp,
                                 bias=negbg_sb[:], scale=-1.0)
            nc.vector.tensor_scalar_add(ef[:], ef[:], 1.0)
            nc.vector.reciprocal(ef[:], ef[:])
            nc.vector.tensor_scalar(ef[:], ef[:], 0.999, 0.01, ALU.min, ALU.max)
            nc.scalar.activation(logfT[:], ef[:], AF.Ln)

            # k chunk 0 keeps the PE busy while the gate DVE chain runs
            kvp = [None] * NCH
            kps0 = pa.tile([128, C], f32, name="kps", tag="kv")
            for kt in range(KT):
                lhs = xtb_sb[:, kt, bass.ts(0, L)]
                st, sp_ = (kt == 0), (kt == KT - 1)
                nc.tensor.matmul(kps0[:, 0:512], lhs,
                                 wkv_sb[:, kt, 0:512], start=st, stop=sp_)
                nc.tensor.matmul(kps0[:, 512:1024], lhs,
                                 wkv_sb[:, kt, 512:1024], start=st, stop=sp_)

            # ---- batched gate chain: per-chunk cumsums/totals in 7 matmuls
            lgp = gp.tile([128, NCH, H], f32, name="lgp", tag="gsm")
            for ch in range(NCH):
                nc.tensor.matmul(lgp[:, ch, :], logfT[:, bass.ts(ch, L)],
                                 eyef_sb[0:H, 0:H], is_transpose=True,
                                 start=(ch == 0), stop=(ch == NCH - 1))
            logf_sb = small.tile([128, NCH, H], f32, name="logf_sb", tag="logf")
            if has_mask:
                nc.vector.tensor_mul(
                    logf_sb[:], lgp[:],
                    mkc_sb[:, :, None].to_broadcast([128, NCH, H]))
            else:
                nc.vector.tensor_copy(logf_sb[:], lgp[:])
            logf_fl = logf_sb.rearrange("p c h -> p (c h)")
            gps = gp.tile([128, NCH * H], f32, name="gps", tag="gsm")
            nc.tensor.matmul(gps[:], triu_sb[:], logf_fl,
                             start=True, stop=True)
            glp = gp.tile([1, NCH * H], f32, name="glp", tag="gsm")
            nc.tensor.matmul(glp[:], one_ap[:, 0:1], logf_fl,
                             start=True, stop=True)
            # v chunk 0 on the PE while the gate scalar/vector chain runs
            vps0 = pa.tile([128, C], f32, name="vps", tag="kv")
            for kt in range(KT):
                lhs = xtb_sb[:, kt, bass.ts(0, L)]
                st, sp_ = (kt == 0), (kt == KT - 1)
                nc.tensor.matmul(vps0[:, 0:512], lhs,
                                 wkv_sb[:, kt, 1024:1536], start=st, stop=sp_)
                nc.tensor.matmul(vps0[:, 512:1024], lhs,
                                 wkv_sb[:, kt, 1536:2048], start=st, stop=sp_)
            a_c = small.tile([128, NCH, H], f32, name="a_c", tag="a_c")
            nc.vector.tensor_scalar_max(
                a_c.rearrange("p c h -> p (c h)"), gps[:], G_CLAMP)
            ainv = small.tile([128, NCH, H], f32, name="ainv", tag="ainv")
            nc.scalar.activation(ainv[:], a_c[:], AF.Exp, scale=-1.0)
            gl_sb = small.tile([1, 2, NCH * H], f32, name="gl_sb", tag="gl")
            nc.vector.tensor_scalar_max(gl_sb[:, 0, :], glp[:], G_CLAMP)
            egl = small.tile([1, 2, NCH * H], f32, name="egl", tag="egl")
            nc.scalar.activation(egl[:, 0, :], gl_sb[:, 0, :], AF.Exp)
            nc.scalar.activation(egl[:, 1, :], gl_sb[:, 0, :], AF.Exp,
                                 scale=-1.0)
            bca = gp.tile([128, 2 * NCH * H], f32, name="bca", tag="gsm")
            nc.tensor.matmul(bca[:], ones_row[:],
                             egl.rearrange("o s x -> o (s x)"),
                             start=True, stop=True)
            bca_sb = small.tile([128, 2 * NCH * H], f32, name="bca_sb",
                                tag="bca_sb")
            nc.vector.tensor_copy(bca_sb[:], bca[:])
            bfw = bca_sb.rearrange("p (s c h) -> p s c h", s=2, c=NCH)
            # running cross-chunk inverse products (clamped), k scales
            nc.scalar.copy(biall[:, 0, :], ainv[:, 0, :])
            nc.vector.tensor_mul(biall[:, 1, :], ainv[:, 1, :], bfw[:, 1, 0, :])
            bic = small.tile([128, 2, H], f32, name="bic", tag="bic")
            nc.vector.tensor_mul(bic[:, 0, :], bfw[:, 1, 0, :], bfw[:, 1, 1, :])
            nc.vector.tensor_scalar_min(bic[:, 0, :], bic[:, 0, :], 1e26)
            nc.vector.tensor_mul(biall[:, 2, :], ainv[:, 2, :], bic[:, 0, :])
            nc.vector.tensor_mul(bic[:, 1, :], bic[:, 0, :], bfw[:, 1, 2, :])
            nc.vector.tensor_scalar_min(bic[:, 1, :], bic[:, 1, :], 1e26)
            nc.vector.tensor_mul(biall[:, 3, :], ainv[:, 3, :], bic[:, 1, :])
            if has_mask:
                for ch in range(NCH):
                    nc.vector.tensor_mul(
                        biall[:, ch, :], biall[:, ch, :],
                        mkc_sb[:, ch:ch + 1].to_broadcast([128, H]))
            # full-quarter forward product brun; its bf16 copy goes into
            # the collective payload tile early, off the critical path
            br2 = small.tile([128, 2, H], f32, name="br2", tag="br2")
            nc.vector.tensor_mul(br2[:, 0, :], bfw[:, 0, 0, :], bfw[:, 0, 1, :])
            nc.vector.tensor_mul(br2[:, 1, :], bfw[:, 0, 2, :], bfw[:, 0, 3, :])
            brun = keep.tile([128, H], f32, name="brun")
            nc.vector.tensor_mul(brun[:], br2[:, 0, :], br2[:, 1, :])
            seffx = keep.tile([64, H * SE + H], bf16, name="seffx")
            nc.scalar.copy(seffx[:, H * SE:], brun[0:64, :])

            # ---- remaining chunks: kv proj / rope / state deltas interleaved
            kv_post(kps0, vps0, 0)
            kvp[1] = kv_proj(1, split=True)
            u_delta(0)
            kv_post(*kvp[1], 1)
            kvp[2] = kv_proj(2, split=True)
            u_delta(1)
            kv_post(*kvp[2], 2)
            kvp[3] = kv_proj(3, split=True)
            u_delta(2)
            kv_post(*kvp[3], 3)
            u_delta(3)

            # ------------------------------------- summary + AllGather (bf16)
            nc.vector.tensor_mul(
                seffx[:, 0:H * SE].rearrange("p (h e) -> p h e", h=H),
                u_tiles[-1][:],
                brun[0:64, :, None].to_broadcast([64, H, SE]))

            cc_in = dram.tile([64, H * SE + H], bf16, name="cc_in")
            cc_out = dram.tile([256, H * SE + H], bf16, name="cc_out")
            nc.sync.dma_start(cc_in[:], seffx[:])
            nc.gpsimd.collective_compute(
                "AllGather", ALU.bypass,
                replica_groups=[[0, 1, 2, 3], [4, 5, 6, 7]],
                ins=[cc_in.opt()], outs=[cc_out.opt()])
            peers_all = keep.tile([64, 4, H * SE + H], bf16, name="peers")
            nc.sync.dma_start(
                peers_all[:], cc_out.rearrange("(q p) c -> p q c", q=4))
            peers = [peers_all[:, q, :] for q in range(4)]

            # ------------------------- pass B: q, transposes, intra-chunk attn
            phA1.close()
            tp = phA.enter_context(tc.tile_pool(name="tp", bufs=2, space="PSUM"))
            ap_ = phA.enter_context(tc.tile_pool(name="ap", bufs=1, space="PSUM"))
            for ch in range(NCH):
                qps, _ = kv_proj(ch, qmode=True)
                rr = rope_phi(qps, ch, "q")
                rrf = rr.rearrange("p h d -> p (h d)")
                if has_mask:
                    nc.vector.tensor_mul(
                        rrf, rrf,
                        mkc_sb[:, ch:ch + 1].to_broadcast([128, H * D]))

                # transpose q (rr) and k (ktm) to D-major [64, H, 128]
                qt_c = qtp.tile([64, H, L], bf16, name=f"qt{ch}", tag="qt")
                kt_c = work.tile([64, H, L], bf16, name="kt_c", tag="kt_c")
                ktmf = ktm_tiles[ch].rearrange("p h d -> p (h d)")
                for which, (srcf, dst) in enumerate(((rrf, qt_c), (ktmf, kt_c))):
                    for tg in range(4):
                        tps = tp.tile([64, 4, L], bf16, name="tps", tag="tps")
                        for j in range(4):
                            h = tg * 4 + j
                            nc.tensor.matmul(
                                tps[:, j, :], srcf[:, bass.ts(h, 64)],
                                eye_sb[:], is_transpose=True,
                                start=(j == 0), stop=(j == 3))
                        if tg % 2 == 0:
                            nc.vector.tensor_copy(dst[:, tg * 4:(tg + 1) * 4, :],
                                                  tps[:])
                        else:
                            nc.scalar.copy(dst[:, tg * 4:(tg + 1) * 4, :], tps[:])

                # causal intra-chunk attention numerators
                ni_c = nip.tile([128, H, SE], bf16, name=f"ni{ch}", tag="ni")
                for g in range(4):
                    atp = ap_.tile([128, 512], f32, name="atp", tag="atp")
                    for j in range(4):
                        h = 4 * g + j
                        nc.tensor.matmul(atp[:, bass.ts(j, 128)],
                                         kt_c[:, h, :], qt_c[:, h, :],
                                         start=(j == 0), stop=(j == 3))
                    atm = work.tile([128, 4, 128], bf16, name="atm", tag="atm")
                    nc.vector.tensor_mul(
                        atm[:], atp.rearrange("p (j t) -> p j t", j=4),
                        triu_sb[:, None, :].to_broadcast([128, 4, 128]))
                    nps = ap_.tile([128, 4, SE], f32, name="nps", tag="nps")
                    for j in range(4):
                        h = 4 * g + j
                        nc.tensor.matmul(nps[:, j, :], atm[:, j, :],
                                         vext_tiles[ch][:, h, :],
                                         start=(j == 0),
                                         stop=(j == 3 and ch == 0))
                    if ch > 0:
                        # inter-chunk partial q @ U_{ch-1}: collective-free,
                        # so it runs here under the AllGather instead of in
                        # phase 2
                        for j in range(4):
                            h = 4 * g + j
                            nc.tensor.matmul(nps[:, j, :], qt_c[:, h, :],
                                             u16_tiles[ch - 1][:, h, :],
                                             start=False, stop=(j == 3))
                    if g % 2 == 0:
                        nc.scalar.copy(ni_c[:, 4 * g:4 * g + 4, :], nps[:])
                    else:
                        nc.vector.tensor_copy(ni_c[:, 4 * g:4 * g + 4, :],
                                              nps[:])
                qt_tiles.append(qt_c)
                ni_tiles.append(ni_c)

            # combine quarter states: sstart = sum_q S_q * W_q with
            # W_q = sel_q * prod_{q'>q} (A_q' sel_q' + 1-sel_q'); the bf16
            # result is duplicated onto partitions 64:128 for row packing
            sstart = keep.tile([64, H, SE], f32, name="sstart")
            sst16 = keep.tile([64, H, SE], bf16, name="sst16")
            prodP = small.tile([64, H], f32, name="prodP", tag="prodP")
            nc.vector.tensor_copy(prodP[:], one_ap[0:64].to_broadcast([64, H]))
            pv = {}
            for q in (3, 2, 1, 0):
                wq_ = small.tile([64, H], f32, name=f"wq{q}", tag=f"wq{q}")
                nc.vector.tensor_mul(
                    wq_[:], prodP[:],
                    sel_sb[0:64, q:q + 1].to_broadcast([64, H]))
                pv[q] = wq_
                if q:
                    mq = small.tile([64, H], f32, name=f"mq{q}", tag="mq")
                    nc.vector.tensor_mul(
                        mq[:], peers[q][:, H * SE:],
                        sel_sb[0:64, q:q + 1].to_broadcast([64, H]))
                    nc.vector.tensor_add(
                        mq[:], mq[:],
                        isel_sb[0:64, q:q + 1].to_broadcast([64, H]))
                    nc.vector.tensor_mul(prodP[:], prodP[:], mq[:])
            tmon = work.tile([64, H, SE], f32, name="tmon", tag="kt_c")
            tmtw = work.tile([64, H, SE], f32, name="tmtw", tag="rrk")
            nc.vector.tensor_mul(
                tmon[:],
                peers[0][:, 0:H * SE].rearrange("p (h e) -> p h e", h=H),
                pv[0][:, :, None].to_broadcast([64, H, SE]))
            nc.vector.tensor_mul(
                tmtw[:],
                peers[1][:, 0:H * SE].rearrange("p (h e) -> p h e", h=H),
                pv[1][:, :, None].to_broadcast([64, H, SE]))
            nc.vector.tensor_add(sstart[:], tmon[:], tmtw[:])
            nc.vector.tensor_mul(
                tmon[:],
                peers[2][:, 0:H * SE].rearrange("p (h e) -> p h e", h=H),
                pv[2][:, :, None].to_broadcast([64, H, SE]))
            nc.vector.tensor_mul(
                tmtw[:],
                peers[3][:, 0:H * SE].rearrange("p (h e) -> p h e", h=H),
                pv[3][:, :, None].to_broadcast([64, H, SE]))
            nc.vector.tensor_add(tmon[:], tmon[:], tmtw[:])
            nc.vector.tensor_add(sstart[:], sstart[:], tmon[:])
            nc.scalar.copy(sst16[:], sstart[:])

        # ---------------------------------------------------------- phase 2
        with contextlib.ExitStack() as ph2:
            wk2 = ph2.enter_context(tc.tile_pool(name="wk2", bufs=2))
            att = ph2.enter_context(tc.tile_pool(name="att", bufs=NCH))
            ip = ph2.enter_context(tc.tile_pool(name="ip", bufs=3, space="PSUM"))
            tp2 = ph2.enter_context(tc.tile_pool(name="tp2", bufs=1, space="PSUM"))
            op = ph2.enter_context(tc.tile_pool(name="op", bufs=2, space="PSUM"))

            # inter-chunk numerators for all chunks first (keeps PE dense);
            # the intra-chunk term ni joins via a DVE add out of PSUM
            attns = []
            for ch in range(NCH):
                attn = att.tile([128, H, D], bf16, name=f"attn{ch}", tag="attn")
                nif = ni_tiles[ch].rearrange("p h e -> p (h e)")
                for g in range(4):
                    ips = ip.tile([128, 4, SE], f32, name="ips", tag="ips")
                    for j in range(4):
                        h = 4 * g + j
                        nc.tensor.matmul(ips[:, j, :], qt_tiles[ch][:, h, :],
                                         sst16[:, h, :],
                                         start=(j == 0), stop=False)
                    nc.tensor.matmul(ips.rearrange("p j e -> p (j e)"),
                                     eye_sb[:],
                                     nif[:, g * 4 * SE:(g + 1) * 4 * SE],
                                     start=False, stop=True)
                    sl4 = slice(4 * g, 4 * g + 4)
                    rden = wk2.tile([128, 4], f32, name="rden", tag="rden")
                    nc.vector.tensor_scalar_max(rden[:], ips[:, :, D], EPS)
                    nc.vector.reciprocal(rden[:], rden[:])
                    nc.vector.tensor_mul(
                        attn[:, sl4, :], ips[:, :, 0:D],
                        rden[:, :, None].to_broadcast([128, 4, D]))
                attns.append(attn)

            for ch in range(NCH):
                # transpose attn -> C-major, then o-proj + residual(+bias)
                attf = attns[ch].rearrange("p h d -> p (h d)")
                at_sb = wk2.tile([128, KT, 128], bf16, name="at_sb", tag="at_sb")
                for tg in range(2):
                    tps = tp2.tile([128, 512], bf16, name="tps2", tag="tps2")
                    for j in range(4):
                        nc.tensor.matmul(
                            tps[:, bass.ts(j, 128)],
                            attf[:, bass.ts(tg * 4 + j, 128)],
                            eye_sb[:], is_transpose=True,
                            start=(j == 0), stop=(j == 3))
                    if tg == 0:
                        nc.vector.tensor_copy(
                            at_sb.rearrange("p k t -> p (k t)")[:, 0:512], tps[:])
                    else:
                        nc.scalar.copy(
                            at_sb.rearrange("p k t -> p (k t)")[:, 512:1024],
                            tps[:])

                ops = op.tile([128, C], f32, name="ops", tag="ops")
                for nh in range(2):
                    nsl = bass.ts(nh, 512)
                    for kt in range(KT):
                        nc.tensor.matmul(ops[:, nsl], at_sb[:, kt, :],
                                         wo_sb[:, kt, nsl],
                                         start=(kt == 0), stop=(kt == KT - 1))

                # residual add off-PE, frees the o-proj PSUM with one reader
                osum = wk2.tile([128, C], f32, name="osum", tag="osum")
                nc.vector.tensor_add(osum[:], ops[:], xr_tiles[ch][:])

                # LayerNorm (Identity/Square/Sqrt live in one act table)
                mus = wk2.tile([128, 4], f32, name="mus", tag="mus")
                scr = wk2.tile([128, C], f32, name="scr", tag="scr")
                nc.scalar.activation(scr[:], osum[:], AF.Identity,
                                     accum_out=mus[:, 0:1])
                nc.vector.tensor_scalar_mul(mus[:, 1:2], mus[:, 0:1], -1.0 / C)
                nc.scalar.activation(scr[:], osum[:], AF.Square,
                                     bias=mus[:, 1:2], accum_out=mus[:, 2:3])
                nc.vector.tensor_scalar(mus[:, 2:3], mus[:, 2:3], 1.0 / C,
                                        LN_EPS, ALU.mult, ALU.add)
                nc.scalar.activation(mus[:, 2:3], mus[:, 2:3], AF.Sqrt)
                nc.vector.reciprocal(mus[:, 2:3], mus[:, 2:3])
                nc.vector.tensor_mul(mus[:, 3:4], mus[:, 1:2], mus[:, 2:3])
                yln = wk2.tile([128, C], bf16, name="yln", tag="scr")
                nc.scalar.activation(yln[:], osum[:], AF.Identity,
                                     scale=mus[:, 2:3], bias=mus[:, 3:4])
                if has_ln:
                    nc.vector.tensor_mul(yln[:], yln[:], lnw_sb[:])
                    nc.vector.tensor_add(yln[:], yln[:], lnb_sb[:])
                nc.sync.dma_start(P["y"][bass.ts(ch, 128), :], yln[:])

    if SPLIT_WAITS:
        _split_multi_waits(nc)
    return nc


# ---------------------------------------------------------------- host side
def _rope_tables():
    half = D // 2
    inv = 1.0 / (ROPE_BASE ** (np.arange(half, dtype=np.float64) / half))
    t = np.arange(T, dtype=np.float64)
    fr = t[:, None] * inv[None, :]
    cos, sin = np.cos(fr), np.sin(fr)
    out = np.zeros((T, 128), np.float32)
    out[:, 0:32] = cos
    out[:, 32:64] = cos
    out[:, 64:96] = -sin
    out[:, 96:128] = sin
    return out


def _ktile(w, dtype=np.float32):  # [C, N] -> [128, KT, N]
    return np.ascontiguousarray(
        w.reshape(KT, 128, w.shape[1]).transpose(1, 0, 2)).astype(dtype)


_cache = {}
RUN_KW = {}      # extra kwargs for run_bass_kernel_spmd (test harness profiling)
LAST = None      # last BassKernelResults (test harness reads exec_time_ns)


def kernel(x, mask, Wq, Wk, Wv, Wg, bg, Wo, bo, ln_w, ln_b):
    bfl = ml_dtypes.bfloat16
    x = np.asarray(x, np.float32)
    mask = np.asarray(mask)
    has_mask = not np.all(mask == 1)
    has_ln = not (np.all(np.asarray(ln_w) == 1) and np.all(np.asarray(ln_b) == 0))

    key = (has_mask, has_ln)
    if key not in _cache:
        _cache[key] = build(has_mask, has_ln)
    nc = _cache[key]

    wkv = _ktile(np.concatenate(
        [np.asarray(Wk).T, np.asarray(Wv).T], axis=1), bfl)
    wq_t = _ktile(np.ascontiguousarray(np.asarray(Wq).T), bfl)
    wg_t = _ktile(np.ascontiguousarray(np.asarray(Wg, np.float32).T), bfl)
    wo_t = _ktile(np.ascontiguousarray(np.asarray(Wo).T), bfl)
    ropec_full = _rope_tables()
    triu = np.triu(np.ones((128, 128), np.float32))
    eye = np.eye(128)
    onesrow = np.ones((1, 128), np.float32)
    negbg = -np.asarray(bg, np.float32)[:, None]
    bo_f = np.asarray(bo, np.float32)

    in_maps = []
    for c in range(NCORE):
        b, r = c // 4, c % 4
        rows = slice(r * QT, (r + 1) * QT)
        xq = np.ascontiguousarray(x[b, rows].T)   # [C, QT]
        m = {
            "xTb": _ktile(xq, bfl),
            "wg": wg_t,
            "wkv": wkv,
            "wq": wq_t,
            "wo": wo_t,
            "xrows": (np.ascontiguousarray(x[b, rows]) + bo_f[None, :]).astype(bfl),
            "ropec": np.ascontiguousarray(
                ropec_full[rows].reshape(NCH, 128, 128).transpose(1, 0, 2)),
            "triu": triu,
            "eye": eye.astype(bfl),
            "eyef": eye.astype(np.float32),
            "onesrow": onesrow,
            "negbg": negbg,
        }
        sel = np.zeros((128, 4), np.float32)
        sel[:, 0:r] = 1.0
        m["sel"] = sel
        m["isel"] = 1.0 - sel
        if has_mask:
            mk = np.asarray(mask[b, rows], np.float32)
            m["mkc"] = np.ascontiguousarray(mk.reshape(NCH, 128).T)
            m["mki"] = 1.0 - m["mkc"]
        if has_ln:
            m["lnw"] = np.tile(np.asarray(ln_w, np.float32), (128, 1))
            m["lnb"] = np.tile(np.asarray(ln_b, np.float32), (128, 1))
        in_maps.append(m)

    res = run_bass_kernel_spmd(nc, in_maps, list(range(NCORE)), **RUN_KW)
    globals()["LAST"] = res
    out = np.empty((B, T, C), np.float32)
    for c in range(NCORE):
        b, r = c // 4, c % 4
        out[b, r * QT:(r + 1) * QT, :] = res.results[c]["y"]
    return out
